# revision 7
# baseline (speedup 1.0000x reference)
"""Trainium2 Bass kernel for nn_ButterflyFFT (Monarch butterfly, N=4096, B=8192).

Math (per batch row b, viewing x[b] as a 64x64 matrix X with X[p,k]=x[b,p*64+k]):
  stage 1: for each column k: Y[:,k] = w1c[k] @ X[:,k]       (64x64 complex, X real)
  stage 2: for each row    l: Z[l,:] = w2c[l] @ Y[l,:]       (64x64 complex)
  output:  out[b, s*64+l] = Z[l,s]                            (complex64)

Device pipeline per core (B_core=1024, supertiles of BT=256):
  1. gather-DMA x -> T1[(h,p), (b0,k)] fp16 (cast in SWDGE DMA)
  2. stage 1, data-stationary fp16 matmuls: out (b, q2) -> G[b, ch, q*128+c*64+r]
  3. PE transpose per (l, ch): G-slice (b, (c r)) -> T2 (rc, b)
  4. stage 2, weights-stationary fp16 matmuls: O2 (c's*64+s, b)
  5. DMA out fp16 (l, cs, b); host reassembles complex64.
"""

import numpy as np

N = 4096
B = 8192
NCORES = 8
B_CORE = B // NCORES  # 1024
BT = 256              # supertile batch
NT = B_CORE // BT     # 4 supertiles
F16 = np.float16


def _build_host_weights(w1_bfly: np.ndarray, w2_bfly: np.ndarray):
    """W1all[64h+p, k*128 + c*64 + q] = w1_bfly[k,q,p,c]  (dup across h)
       W2all[c*64+r, l*128 + c'*64 + s] = stage-2 complex-matmul real form."""
    w1 = w1_bfly.astype(np.float32)              # (k, q, p, c)
    W1 = np.transpose(w1, (2, 0, 3, 1))          # (p, k, c, q)
    W1 = W1.reshape(64, 64 * 128).astype(F16)    # [p, k*128 + c*64 + q]
    W1all = np.concatenate([W1, W1], axis=0)     # dup rows for h=0/1

    w2r = w2_bfly[..., 0].astype(np.float32)     # (l, s, r)
    w2i = w2_bfly[..., 1].astype(np.float32)
    W2 = np.empty((2, 64, 64, 2, 64), dtype=np.float32)  # [c, r, l, c', s]
    W2[0, :, :, 0, :] = np.transpose(w2r, (2, 0, 1))     # rows r,    out re:  w2_re
    W2[1, :, :, 0, :] = -np.transpose(w2i, (2, 0, 1))    # rows 64+r, out re: -w2_im
    W2[0, :, :, 1, :] = np.transpose(w2i, (2, 0, 1))     # rows r,    out im:  w2_im
    W2[1, :, :, 1, :] = np.transpose(w2r, (2, 0, 1))     # rows 64+r, out im:  w2_re
    W2all = W2.reshape(128, 64 * 128).astype(F16)        # [c*64+r, l*128 + c'*64 + s]
    return np.ascontiguousarray(W1all), np.ascontiguousarray(W2all)


def build_bass():
    import concourse.bacc as bacc
    import concourse.mybir as mybir
    import concourse.tile as tile

    f16 = mybir.dt.float16
    f32 = mybir.dt.float32

    nc = bacc.Bacc("TRN2", target_bir_lowering=False)
    x = nc.dram_tensor("x", [B_CORE, N], f32, kind="ExternalInput")
    w1 = nc.dram_tensor("w1", [128, 64 * 128], f16, kind="ExternalInput")
    w2 = nc.dram_tensor("w2", [128, 64 * 128], f16, kind="ExternalInput")
    iddram = nc.dram_tensor("ident", [128, 128], f16, kind="ExternalInput")
    out = nc.dram_tensor("out", [64, 128, B_CORE], f16, kind="ExternalOutput")

    x_v = x[:, :].rearrange("(t h b0) (p k) -> t h p b0 k", h=2, b0=BT // 2, p=64)
    out_v = out[:, :, :].rearrange("L cs (t b) -> t cs L b", b=BT)

    with tile.TileContext(nc) as tc:
        with (
            tc.tile_pool(name="const", bufs=1) as constp,
            tc.tile_pool(name="t1", bufs=2) as t1p,
            tc.tile_pool(name="g", bufs=2) as gp,
            tc.tile_pool(name="t2s", bufs=4) as t2p,
            tc.tile_pool(name="outs", bufs=2) as outp,
            tc.tile_pool(name="po1", bufs=2, space="PSUM") as po1,
            tc.tile_pool(name="pt2", bufs=2, space="PSUM") as pt2,
            tc.tile_pool(name="po2", bufs=2, space="PSUM") as po2,
        ):
            W1t = constp.tile([128, 64 * 128], f16)
            nc.sync.dma_start(W1t[:], w1[:, :])
            W2t = constp.tile([128, 64 * 128], f16)
            nc.sync.dma_start(W2t[:], w2[:, :])
            ident = constp.tile([128, 128], f16)
            nc.sync.dma_start(ident[:], iddram[:, :])
            W1t_v = W1t[:].rearrange("(h p) f -> h p f", h=2)

            for t in range(NT):
                # ---- load T1[(h,p), (b0,k)] with fp32->fp16 cast (SWDGE) ----
                T1 = t1p.tile([128, (BT // 2) * 64], f16)
                T1_4d = T1[:].rearrange("(h p) (b0 k) -> h p b0 k", h=2, k=64)
                for h in range(2):
                    nc.gpsimd.dma_start(T1_4d[h], x_v[t][h])

                # ---- stage 1 (data-stationary): G[b, ch, q*128 + c*64 + r] ----
                G = gp.tile([128, 2, 64 * 128], f16)
                G_5d = G[:].rearrange("B ch (q c r) -> B ch q c r", q=64, c=2)
                for ch in range(2):
                    for kg in range(16):
                        O1 = po1.tile([128, 512], f32)
                        for ksub in range(4):
                            k = kg * 4 + ksub
                            nc.tensor.matmul(
                                O1[:, ksub * 128:(ksub + 1) * 128],
                                T1_4d[ch][:, :, k],                  # (64 p, 128 b0)
                                W1t_v[ch][:, k * 128:(k + 1) * 128], # (64 p, 128 q2)
                                start=True, stop=True,
                            )
                        # evac + cast f32->f16 on ACT, scatter (ksub,c,q)->(q,c,ksub)
                        src = O1[:].rearrange("B (ksub c q) -> B q c ksub", ksub=4, c=2)
                        dst = G_5d[:, ch, :, :, kg * 4:(kg + 1) * 4]
                        nc.scalar.copy(dst, src)

                # ---- stage 2 per l: PE transpose then matmul ----
                OUTS = None
                for l in range(64):
                    grp = (t * 64 + l) // 16
                    if l % 16 == 0:
                        OUTS = outp.tile([128, 16, BT], f16)

                    Pt2 = pt2.tile([128, 2, 128], f16)
                    for ch in range(2):
                        nc.tensor.transpose(
                            Pt2[:, ch, :], G[:, ch, l * 128:(l + 1) * 128], ident[:]
                        )
                    T2s = t2p.tile([128, 256], f16)
                    nc.vector.tensor_copy(T2s[:], Pt2[:])

                    O2 = po2.tile([128, BT], f32)
                    nc.tensor.matmul(
                        O2[:], W2t[:, l * 128:(l + 1) * 128], T2s[:],
                        start=True, stop=True,
                    )
                    if grp % 2 == 0:
                        nc.scalar.copy(OUTS[:, l % 16, :], O2[:])
                    else:
                        nc.vector.tensor_copy(OUTS[:, l % 16, :], O2[:])

                    if l % 16 == 15:
                        nc.sync.dma_start(out_v[t][:, l - 15:l + 1, :], OUTS[:])
    nc.compile()
    return nc


def _assemble_core(o: np.ndarray) -> np.ndarray:
    # o: (64 l, 128 cs, B_CORE) f16, cs = c*64+s  ->  (B_CORE, 4096) complex64
    a = o.reshape(64, 2, 64, B_CORE)                         # (l, c, s, b)
    a = np.ascontiguousarray(np.transpose(a, (3, 2, 0, 1)))  # (b, s, l, c)
    return a.astype(np.float32).view(np.complex64).reshape(B_CORE, N)


def kernel(x, w1_bfly, w2_bfly, perm, _trace=False):
    from concourse.bass_utils import run_bass_kernel_spmd

    x = np.asarray(x, dtype=np.float32)
    w1_bfly = np.asarray(w1_bfly, dtype=np.float32)
    w2_bfly = np.asarray(w2_bfly, dtype=np.float32)

    W1all, W2all = _build_host_weights(w1_bfly, w2_bfly)
    ident = np.eye(128, dtype=F16)
    nc = build_bass()
    in_maps = [
        {
            "x": np.ascontiguousarray(x[i * B_CORE:(i + 1) * B_CORE]),
            "w1": W1all,
            "w2": W2all,
            "ident": ident,
        }
        for i in range(NCORES)
    ]
    res = run_bass_kernel_spmd(
        nc, in_maps, core_ids=list(range(NCORES)), trace=_trace
    )
    outs = [_assemble_core(r["out"]) for r in res.results]
    full = np.concatenate(outs, axis=0)
    if _trace:
        return full, res
    return full


# revision 10
# speedup vs baseline: 302.4531x; 302.4531x over previous
"""Trainium2 Bass kernel for nn_ButterflyFFT (Monarch butterfly, N=4096, B=8192).

Math (per batch row b, viewing x[b] as a 64x64 matrix X with X[p,k]=x[b,p*64+k]):
  stage 1: for each column k: Y[:,k] = w1c[k] @ X[:,k]       (64x64 complex, X real)
  stage 2: for each row    l: Z[l,:] = w2c[l] @ Y[l,:]       (64x64 complex)
  output:  out[b, s*64+l] = Z[l,s]                            (complex64)

Device pipeline per core (B_core=1024, supertiles of BT=256):
  1. gather-DMA x -> T1[(h,p), (b0,k)] fp16 (cast in SWDGE DMA)
  2. stage 1, data-stationary fp16 matmuls: out (b, q2) -> G[b, ch, q*128+c*64+r]
  3. PE transpose per (l, ch): G-slice (b, (c r)) -> T2 (rc, b)
  4. stage 2, weights-stationary fp16 matmuls: O2 (c's*64+s, b)
  5. DMA out fp16 (l, cs, b); host reassembles complex64.
"""

import numpy as np

N = 4096
B = 8192
NCORES = 8
B_CORE = B // NCORES  # 1024
BT = 256              # supertile batch
NT = B_CORE // BT     # 4 supertiles
F16 = np.float16


def _build_host_weights(w1_bfly: np.ndarray, w2_bfly: np.ndarray):
    """W1all[64h+p, k*128 + c*64 + q] = w1_bfly[k,q,p,c]  (dup across h)
       W2all[c*64+r, l*128 + c'*64 + s] = stage-2 complex-matmul real form."""
    w1 = w1_bfly.astype(np.float32)              # (k, q, p, c)
    W1 = np.transpose(w1, (2, 0, 3, 1))          # (p, k, c, q)
    W1 = W1.reshape(64, 64 * 128).astype(F16)    # [p, k*128 + c*64 + q]
    W1all = np.concatenate([W1, W1], axis=0)     # dup rows for h=0/1

    w2r = w2_bfly[..., 0].astype(np.float32)     # (l, s, r)
    w2i = w2_bfly[..., 1].astype(np.float32)
    W2 = np.empty((2, 64, 64, 2, 64), dtype=np.float32)  # [c, r, l, c', s]
    W2[0, :, :, 0, :] = np.transpose(w2r, (2, 0, 1))     # rows r,    out re:  w2_re
    W2[1, :, :, 0, :] = -np.transpose(w2i, (2, 0, 1))    # rows 64+r, out re: -w2_im
    W2[0, :, :, 1, :] = np.transpose(w2i, (2, 0, 1))     # rows r,    out im:  w2_im
    W2[1, :, :, 1, :] = np.transpose(w2r, (2, 0, 1))     # rows 64+r, out im:  w2_re
    W2all = W2.reshape(128, 64 * 128).astype(F16)        # [c*64+r, l*128 + c'*64 + s]
    return np.ascontiguousarray(W1all), np.ascontiguousarray(W2all)


def build_bass(repeat=1):
    import concourse.bacc as bacc
    import concourse.mybir as mybir
    import concourse.tile as tile

    f16 = mybir.dt.float16
    f32 = mybir.dt.float32

    nc = bacc.Bacc("TRN2", target_bir_lowering=False)
    x = nc.dram_tensor("x", [B_CORE, N], f32, kind="ExternalInput")
    w1 = nc.dram_tensor("w1", [128, 64 * 128], f16, kind="ExternalInput")
    w2 = nc.dram_tensor("w2", [128, 64 * 128], f16, kind="ExternalInput")
    iddram = nc.dram_tensor("ident", [128, 128], f16, kind="ExternalInput")
    out = nc.dram_tensor("out", [64, 128, B_CORE], f16, kind="ExternalOutput")

    x_v = x[:, :].rearrange("(t h b0) (p k) -> t h p b0 k", h=2, b0=BT // 2, p=64)
    out_v = out[:, :, :].rearrange("L cs (t b) -> t cs L b", b=BT)

    with tile.TileContext(nc) as tc:
        with (
            tc.tile_pool(name="const", bufs=1) as constp,
            tc.tile_pool(name="t1", bufs=2) as t1p,
            tc.tile_pool(name="g", bufs=2) as gp,
            tc.tile_pool(name="t2s", bufs=4) as t2p,
            tc.tile_pool(name="outs", bufs=2) as outp,
            tc.tile_pool(name="po1", bufs=2, space="PSUM") as po1,
            tc.tile_pool(name="pt2", bufs=2, space="PSUM") as pt2,
            tc.tile_pool(name="po2", bufs=2, space="PSUM") as po2,
        ):
            W1t = constp.tile([128, 64 * 128], f16)
            nc.sync.dma_start(W1t[:], w1[:, :])
            W2t = constp.tile([128, 64 * 128], f16)
            nc.sync.dma_start(W2t[:], w2[:, :])
            ident = constp.tile([128, 128], f16)
            nc.sync.dma_start(ident[:], iddram[:, :])
            W1t_v = W1t[:].rearrange("(h p) f -> h p f", h=2)

            from contextlib import nullcontext
            rep_ctx = tc.For_i(0, repeat, 1) if repeat > 1 else nullcontext()
            with rep_ctx:
                for t in range(NT):
                    # ---- load T1[(h,p), (b0,k)] with fp32->fp16 cast (SWDGE) ----
                    T1 = t1p.tile([128, (BT // 2) * 64], f16)
                    T1_4d = T1[:].rearrange("(h p) (b0 k) -> h p b0 k", h=2, k=64)
                    for h in range(2):
                        nc.gpsimd.dma_start(T1_4d[h], x_v[t][h])

                    # ---- stage 1 (data-stationary): G[b, ch, q*128+c*64+r] ----
                    G = gp.tile([128, 2, 64 * 128], f16)
                    G_5d = G[:].rearrange("B ch (q c r) -> B ch q c r", q=64, c=2)
                    for ch in range(2):
                        for kg in range(16):
                            O1 = po1.tile([128, 512], f32)
                            for ksub in range(4):
                                k = kg * 4 + ksub
                                nc.tensor.matmul(
                                    O1[:, ksub * 128:(ksub + 1) * 128],
                                    T1_4d[ch][:, :, k],                  # (64 p, 128 b0)
                                    W1t_v[ch][:, k * 128:(k + 1) * 128], # (64 p, 128 q2)
                                    start=True, stop=True,
                                )
                            # evac + cast f32->f16 on ACT, (ksub,c,q)->(q,c,ksub)
                            src = O1[:].rearrange(
                                "B (ksub c q) -> B q c ksub", ksub=4, c=2)
                            dst = G_5d[:, ch, :, :, kg * 4:(kg + 1) * 4]
                            nc.scalar.copy(dst, src)

                    # ---- stage 2 per l: PE transpose then matmul ----
                    OUTS = None
                    for l in range(64):
                        grp = (t * 64 + l) // 16
                        if l % 16 == 0:
                            OUTS = outp.tile([128, 16, BT], f16)

                        Pt2 = pt2.tile([128, 2, 128], f16)
                        for ch in range(2):
                            nc.tensor.transpose(
                                Pt2[:, ch, :], G[:, ch, l * 128:(l + 1) * 128],
                                ident[:]
                            )
                        T2s = t2p.tile([128, 256], f16)
                        nc.vector.tensor_copy(T2s[:], Pt2[:])

                        O2 = po2.tile([128, BT], f32)
                        nc.tensor.matmul(
                            O2[:], W2t[:, l * 128:(l + 1) * 128], T2s[:],
                            start=True, stop=True,
                        )
                        if grp % 2 == 0:
                            nc.scalar.copy(OUTS[:, l % 16, :], O2[:])
                        else:
                            nc.vector.tensor_copy(OUTS[:, l % 16, :], O2[:])

                        if l % 16 == 15:
                            nc.sync.dma_start(out_v[t][:, l - 15:l + 1, :], OUTS[:])
    nc.compile()
    return nc


def _assemble_core(o: np.ndarray) -> np.ndarray:
    # o: (64 l, 128 cs, B_CORE) f16, cs = c*64+s  ->  (B_CORE, 4096) complex64
    a = o.reshape(64, 2, 64, B_CORE)                         # (l, c, s, b)
    a = np.ascontiguousarray(np.transpose(a, (3, 2, 0, 1)))  # (b, s, l, c)
    return a.astype(np.float32).view(np.complex64).reshape(B_CORE, N)


def kernel(x, w1_bfly, w2_bfly, perm, _trace=False):
    from concourse.bass_utils import run_bass_kernel_spmd

    x = np.asarray(x, dtype=np.float32)
    w1_bfly = np.asarray(w1_bfly, dtype=np.float32)
    w2_bfly = np.asarray(w2_bfly, dtype=np.float32)

    W1all, W2all = _build_host_weights(w1_bfly, w2_bfly)
    ident = np.eye(128, dtype=F16)
    nc = build_bass()
    in_maps = [
        {
            "x": np.ascontiguousarray(x[i * B_CORE:(i + 1) * B_CORE]),
            "w1": W1all,
            "w2": W2all,
            "ident": ident,
        }
        for i in range(NCORES)
    ]
    res = run_bass_kernel_spmd(
        nc, in_maps, core_ids=list(range(NCORES)), trace=_trace
    )
    outs = [_assemble_core(r["out"]) for r in res.results]
    full = np.concatenate(outs, axis=0)
    if _trace:
        return full, res
    return full


# revision 12
# speedup vs baseline: 305.7409x; 1.0109x over previous
"""Trainium2 Bass kernel for nn_ButterflyFFT (Monarch butterfly, N=4096, B=8192).

Math (per batch row b, viewing x[b] as a 64x64 matrix X with X[p,k]=x[b,p*64+k]):
  stage 1: for each column k: Y[:,k] = w1c[k] @ X[:,k]       (64x64 complex, X real)
  stage 2: for each row    l: Z[l,:] = w2c[l] @ Y[l,:]       (64x64 complex)
  output:  out[b, s*64+l] = Z[l,s]                            (complex64)

Device pipeline per core (B_core=1024, supertiles of BT=256):
  1. gather-DMA x -> T1[(h,p), (b0,k)] fp16 (cast in SWDGE DMA)
  2. stage 1, data-stationary fp16 matmuls: out (b, q2) -> G[b, ch, q*128+c*64+r]
  3. PE transpose per (l, ch): G-slice (b, (c r)) -> T2 (rc, b)
  4. stage 2, weights-stationary fp16 matmuls: O2 (c's*64+s, b)
  5. DMA out fp16 (l, cs, b); host reassembles complex64.
"""

import numpy as np

N = 4096
B = 8192
NCORES = 8
B_CORE = B // NCORES  # 1024
BT = 256              # supertile batch
NT = B_CORE // BT     # 4 supertiles
F16 = np.float16


def _build_host_weights(w1_bfly: np.ndarray, w2_bfly: np.ndarray):
    """W1all[64h+p, k*128 + c*64 + q] = w1_bfly[k,q,p,c]  (dup across h)
       W2all[c*64+r, l*128 + c'*64 + s] = stage-2 complex-matmul real form."""
    w1 = w1_bfly.astype(np.float32)              # (k, q, p, c)
    W1 = np.transpose(w1, (2, 0, 3, 1))          # (p, k, c, q)
    W1 = W1.reshape(64, 64 * 128).astype(F16)    # [p, k*128 + c*64 + q]
    W1all = np.concatenate([W1, W1], axis=0)     # dup rows for h=0/1

    w2r = w2_bfly[..., 0].astype(np.float32)     # (l, s, r)
    w2i = w2_bfly[..., 1].astype(np.float32)
    W2 = np.empty((2, 64, 64, 2, 64), dtype=np.float32)  # [c, r, l, c', s]
    W2[0, :, :, 0, :] = np.transpose(w2r, (2, 0, 1))     # rows r,    out re:  w2_re
    W2[1, :, :, 0, :] = -np.transpose(w2i, (2, 0, 1))    # rows 64+r, out re: -w2_im
    W2[0, :, :, 1, :] = np.transpose(w2i, (2, 0, 1))     # rows r,    out im:  w2_im
    W2[1, :, :, 1, :] = np.transpose(w2r, (2, 0, 1))     # rows 64+r, out im:  w2_re
    W2all = W2.reshape(128, 64 * 128).astype(F16)        # [c*64+r, l*128 + c'*64 + s]
    return np.ascontiguousarray(W1all), np.ascontiguousarray(W2all)


def build_bass(repeat=1):
    import concourse.bacc as bacc
    import concourse.mybir as mybir
    import concourse.tile as tile

    f16 = mybir.dt.float16
    f32 = mybir.dt.float32

    nc = bacc.Bacc("TRN2", target_bir_lowering=False)
    x = nc.dram_tensor("x", [B_CORE, N], f32, kind="ExternalInput")
    w1 = nc.dram_tensor("w1", [128, 64 * 128], f16, kind="ExternalInput")
    w2 = nc.dram_tensor("w2", [128, 64 * 128], f16, kind="ExternalInput")
    iddram = nc.dram_tensor("ident", [128, 128], f16, kind="ExternalInput")
    out = nc.dram_tensor("out", [64, 128, B_CORE], f16, kind="ExternalOutput")

    x_v = x[:, :].rearrange("(t h b0) (p k) -> t h p b0 k", h=2, b0=BT // 2, p=64)
    out_v = out[:, :, :].rearrange("L cs (t b) -> t cs L b", b=BT)

    with tile.TileContext(nc) as tc:
        with (
            tc.tile_pool(name="const", bufs=1) as constp,
            tc.tile_pool(name="t1", bufs=2) as t1p,
            tc.tile_pool(name="g", bufs=2) as gp,
            tc.tile_pool(name="t2s", bufs=4) as t2p,
            tc.tile_pool(name="outs", bufs=3) as outp,
            tc.tile_pool(name="po1", bufs=3, space="PSUM") as po1,
            tc.tile_pool(name="pt2", bufs=2, space="PSUM") as pt2,
            tc.tile_pool(name="po2", bufs=3, space="PSUM") as po2,
        ):
            W1t = constp.tile([128, 64 * 128], f16)
            nc.sync.dma_start(W1t[:], w1[:, :])
            W2t = constp.tile([128, 64 * 128], f16)
            nc.sync.dma_start(W2t[:], w2[:, :])
            ident = constp.tile([128, 128], f16)
            nc.sync.dma_start(ident[:], iddram[:, :])
            W1t_v = W1t[:].rearrange("(h p) f -> h p f", h=2)

            from contextlib import nullcontext
            rep_ctx = tc.For_i(0, repeat, 1) if repeat > 1 else nullcontext()
            with rep_ctx:
                for t in range(NT):
                    # ---- load T1[(h,p), (b0,k)] with fp32->fp16 cast (SWDGE) ----
                    T1 = t1p.tile([128, (BT // 2) * 64], f16)
                    T1_4d = T1[:].rearrange("(h p) (b0 k) -> h p b0 k", h=2, k=64)
                    for h in range(2):
                        nc.gpsimd.dma_start(T1_4d[h], x_v[t][h])

                    # ---- stage 1 (data-stationary): G[b, ch, q*128+c*64+r] ----
                    G = gp.tile([128, 2, 64 * 128], f16)
                    G_5d = G[:].rearrange("B ch (q c r) -> B ch q c r", q=64, c=2)
                    for ch in range(2):
                        for kg in range(16):
                            O1 = po1.tile([128, 512], f32)
                            for ksub in range(4):
                                k = kg * 4 + ksub
                                nc.tensor.matmul(
                                    O1[:, ksub * 128:(ksub + 1) * 128],
                                    T1_4d[ch][:, :, k],                  # (64 p, 128 b0)
                                    W1t_v[ch][:, k * 128:(k + 1) * 128], # (64 p, 128 q2)
                                    start=True, stop=True,
                                )
                            # evac + cast f32->f16 on ACT, (ksub,c,q)->(q,c,ksub)
                            src = O1[:].rearrange(
                                "B (ksub c q) -> B q c ksub", ksub=4, c=2)
                            dst = G_5d[:, ch, :, :, kg * 4:(kg + 1) * 4]
                            nc.scalar.copy(dst, src)

                    # ---- stage 2, l in pairs: PE transposes then matmuls ----
                    OUTS = None
                    for l0 in range(0, 64, 2):
                        grp = (t * 64 + l0) // 16
                        if l0 % 16 == 0:
                            OUTS = outp.tile([128, 16, BT], f16)

                        Pt2 = pt2.tile([128, 4, 128], f16)
                        for lp in range(2):
                            l = l0 + lp
                            for ch in range(2):
                                nc.tensor.transpose(
                                    Pt2[:, lp * 2 + ch, :],
                                    G[:, ch, l * 128:(l + 1) * 128], ident[:]
                                )
                        T2s = t2p.tile([128, 512], f16)
                        nc.vector.tensor_copy(T2s[:], Pt2[:])

                        O2 = po2.tile([128, 2, BT], f32)
                        for lp in range(2):
                            l = l0 + lp
                            nc.tensor.matmul(
                                O2[:, lp, :], W2t[:, l * 128:(l + 1) * 128],
                                T2s[:, lp * 256:(lp + 1) * 256],
                                start=True, stop=True,
                            )
                        if grp % 2 == 0:
                            nc.scalar.copy(OUTS[:, l0 % 16:l0 % 16 + 2, :], O2[:])
                        else:
                            nc.vector.tensor_copy(OUTS[:, l0 % 16:l0 % 16 + 2, :], O2[:])

                        if l0 % 16 == 14:
                            nc.sync.dma_start(out_v[t][:, l0 - 14:l0 + 2, :], OUTS[:])
    nc.compile()
    return nc


def _assemble_core(o: np.ndarray) -> np.ndarray:
    # o: (64 l, 128 cs, B_CORE) f16, cs = c*64+s  ->  (B_CORE, 4096) complex64
    a = o.reshape(64, 2, 64, B_CORE)                         # (l, c, s, b)
    a = np.ascontiguousarray(np.transpose(a, (3, 2, 0, 1)))  # (b, s, l, c)
    return a.astype(np.float32).view(np.complex64).reshape(B_CORE, N)


def kernel(x, w1_bfly, w2_bfly, perm, _trace=False):
    from concourse.bass_utils import run_bass_kernel_spmd

    x = np.asarray(x, dtype=np.float32)
    w1_bfly = np.asarray(w1_bfly, dtype=np.float32)
    w2_bfly = np.asarray(w2_bfly, dtype=np.float32)

    W1all, W2all = _build_host_weights(w1_bfly, w2_bfly)
    ident = np.eye(128, dtype=F16)
    nc = build_bass()
    in_maps = [
        {
            "x": np.ascontiguousarray(x[i * B_CORE:(i + 1) * B_CORE]),
            "w1": W1all,
            "w2": W2all,
            "ident": ident,
        }
        for i in range(NCORES)
    ]
    res = run_bass_kernel_spmd(
        nc, in_maps, core_ids=list(range(NCORES)), trace=_trace
    )
    outs = [_assemble_core(r["out"]) for r in res.results]
    full = np.concatenate(outs, axis=0)
    if _trace:
        return full, res
    return full


# revision 14
# speedup vs baseline: 313.4881x; 1.0253x over previous
"""Trainium2 Bass kernel for nn_ButterflyFFT (Monarch butterfly, N=4096, B=8192).

Math (per batch row b, viewing x[b] as a 64x64 matrix X with X[p,k]=x[b,p*64+k]):
  stage 1: for each column k: Y[:,k] = w1c[k] @ X[:,k]       (64x64 complex, X real)
  stage 2: for each row    l: Z[l,:] = w2c[l] @ Y[l,:]       (64x64 complex)
  output:  out[b, s*64+l] = Z[l,s]                            (complex64)

Device pipeline per core (B_core=1024, supertiles of BT=256):
  1. gather-DMA x -> T1[(h,p), (b0,k)] fp16 (cast in SWDGE DMA)
  2. stage 1, data-stationary fp16 matmuls: out (b, q2) -> G[b, ch, q*128+c*64+r]
  3. PE transpose per (l, ch): G-slice (b, (c r)) -> T2 (rc, b)
  4. stage 2, weights-stationary fp16 matmuls: O2 (c's*64+s, b)
  5. DMA out fp16 (l, cs, b); host reassembles complex64.
"""

import numpy as np

N = 4096
B = 8192
NCORES = 8
B_CORE = B // NCORES  # 1024
BT = 256              # supertile batch
NT = B_CORE // BT     # 4 supertiles
F16 = np.float16


def _build_host_weights(w1_bfly: np.ndarray, w2_bfly: np.ndarray):
    """W1all[64h+p, k*128 + c*64 + q] = w1_bfly[k,q,p,c]  (dup across h)
       W2all[c*64+r, l*128 + c'*64 + s] = stage-2 complex-matmul real form."""
    w1 = w1_bfly.astype(np.float32)              # (k, q, p, c)
    W1 = np.transpose(w1, (2, 0, 3, 1))          # (p, k, c, q)
    W1 = W1.reshape(64, 64 * 128).astype(F16)    # [p, k*128 + c*64 + q]
    W1all = np.concatenate([W1, W1], axis=0)     # dup rows for h=0/1

    w2r = w2_bfly[..., 0].astype(np.float32)     # (l, s, r)
    w2i = w2_bfly[..., 1].astype(np.float32)
    W2 = np.empty((2, 64, 64, 2, 64), dtype=np.float32)  # [c, r, l, c', s]
    W2[0, :, :, 0, :] = np.transpose(w2r, (2, 0, 1))     # rows r,    out re:  w2_re
    W2[1, :, :, 0, :] = -np.transpose(w2i, (2, 0, 1))    # rows 64+r, out re: -w2_im
    W2[0, :, :, 1, :] = np.transpose(w2i, (2, 0, 1))     # rows r,    out im:  w2_im
    W2[1, :, :, 1, :] = np.transpose(w2r, (2, 0, 1))     # rows 64+r, out im:  w2_re
    W2all = W2.reshape(128, 64 * 128).astype(F16)        # [c*64+r, l*128 + c'*64 + s]
    return np.ascontiguousarray(W1all), np.ascontiguousarray(W2all)


def build_bass(repeat=1):
    import concourse.bacc as bacc
    import concourse.mybir as mybir
    import concourse.tile as tile

    f16 = mybir.dt.float16
    f32 = mybir.dt.float32

    nc = bacc.Bacc("TRN2", target_bir_lowering=False)
    x = nc.dram_tensor("x", [B_CORE, N], f32, kind="ExternalInput")
    w1 = nc.dram_tensor("w1", [128, 64 * 128], f16, kind="ExternalInput")
    w2 = nc.dram_tensor("w2", [128, 64 * 128], f16, kind="ExternalInput")
    iddram = nc.dram_tensor("ident", [128, 128], f16, kind="ExternalInput")
    out = nc.dram_tensor("out", [64, 128, B_CORE], f16, kind="ExternalOutput")

    x_v = x[:, :].rearrange("(t h b0) (p k) -> t h p b0 k", h=2, b0=BT // 2, p=64)
    out_v = out[:, :, :].rearrange("L cs (t b) -> t cs L b", b=BT)

    with tile.TileContext(nc) as tc:
        with (
            tc.tile_pool(name="const", bufs=1) as constp,
            tc.tile_pool(name="t1", bufs=2) as t1p,
            tc.tile_pool(name="g", bufs=2) as gp,
            tc.tile_pool(name="t2s", bufs=4) as t2p,
            tc.tile_pool(name="outs", bufs=3) as outp,
            tc.tile_pool(name="po1", bufs=3, space="PSUM") as po1,
            tc.tile_pool(name="pt2", bufs=2, space="PSUM") as pt2,
            tc.tile_pool(name="po2", bufs=3, space="PSUM") as po2,
        ):
            W1t = constp.tile([128, 64 * 128], f16)
            nc.sync.dma_start(W1t[:], w1[:, :])
            W2t = constp.tile([128, 64 * 128], f16)
            nc.sync.dma_start(W2t[:], w2[:, :])
            ident = constp.tile([128, 128], f16)
            nc.sync.dma_start(ident[:], iddram[:, :])
            W1t_v = W1t[:].rearrange("(h p) f -> h p f", h=2)

            from contextlib import nullcontext
            rep_ctx = tc.For_i(0, repeat, 1) if repeat > 1 else nullcontext()
            with rep_ctx:
                for t in range(NT):
                    # ---- load T1[(h,p), (b0,k)] with fp32->fp16 cast (SWDGE) ----
                    T1 = t1p.tile([128, (BT // 2) * 64], f16)
                    T1_4d = T1[:].rearrange("(h p) (b0 k) -> h p b0 k", h=2, k=64)
                    for h in range(2):
                        nc.gpsimd.dma_start(T1_4d[h], x_v[t][h])

                    # ---- stage 1 (data-stationary): G[b, ch, q*128+c*64+r] ----
                    G = gp.tile([128, 2, 64 * 128], f16)
                    G_5d = G[:].rearrange("B ch (q c r) -> B ch q c r", q=64, c=2)
                    for ch in range(2):
                        for kg in range(16):
                            O1 = po1.tile([128, 512], f32)
                            for ksub in range(4):
                                k = kg * 4 + ksub
                                nc.tensor.matmul(
                                    O1[:, ksub * 128:(ksub + 1) * 128],
                                    T1_4d[ch][:, :, k],                  # (64 p, 128 b0)
                                    W1t_v[ch][:, k * 128:(k + 1) * 128], # (64 p, 128 q2)
                                    start=True, stop=True,
                                )
                            # evac + cast f32->f16, (ksub,c,q)->(q,c,ksub)
                            src = O1[:].rearrange(
                                "B (ksub c q) -> B q c ksub", ksub=4, c=2)
                            dst = G_5d[:, ch, :, :, kg * 4:(kg + 1) * 4]
                            if kg % 4 == 3:
                                nc.vector.tensor_copy(dst, src)
                            else:
                                nc.scalar.copy(dst, src)

                    # ---- stage 2, l in quads: PE transposes -> T2s; pairs of mms ----
                    OUTS = None
                    T2s = None
                    for l0 in range(0, 64, 2):
                        grp = (t * 64 + l0) // 16
                        if l0 % 16 == 0:
                            OUTS = outp.tile([128, 16, BT], f16)
                        if l0 % 4 == 0:
                            Pt2 = pt2.tile([128, 8, 128], f16)
                            for lp in range(4):
                                l = l0 + lp
                                for ch in range(2):
                                    nc.tensor.transpose(
                                        Pt2[:, lp * 2 + ch, :],
                                        G[:, ch, l * 128:(l + 1) * 128], ident[:]
                                    )
                            T2s = t2p.tile([128, 4, 256], f16)
                            nc.vector.tensor_copy(T2s[:], Pt2[:])

                        O2 = po2.tile([128, 2, BT], f32)
                        for lp in range(2):
                            l = l0 + lp
                            nc.tensor.matmul(
                                O2[:, lp, :], W2t[:, l * 128:(l + 1) * 128],
                                T2s[:, l % 4, :],
                                start=True, stop=True,
                            )
                        if grp % 2 == 0:
                            nc.scalar.copy(OUTS[:, l0 % 16:l0 % 16 + 2, :], O2[:])
                        else:
                            nc.vector.tensor_copy(OUTS[:, l0 % 16:l0 % 16 + 2, :], O2[:])

                        if l0 % 16 == 14:
                            nc.sync.dma_start(out_v[t][:, l0 - 14:l0 + 2, :], OUTS[:])
    nc.compile()
    return nc


def _assemble_core(o: np.ndarray) -> np.ndarray:
    # o: (64 l, 128 cs, B_CORE) f16, cs = c*64+s  ->  (B_CORE, 4096) complex64
    a = o.reshape(64, 2, 64, B_CORE)                         # (l, c, s, b)
    a = np.ascontiguousarray(np.transpose(a, (3, 2, 0, 1)))  # (b, s, l, c)
    return a.astype(np.float32).view(np.complex64).reshape(B_CORE, N)


def kernel(x, w1_bfly, w2_bfly, perm, _trace=False):
    from concourse.bass_utils import run_bass_kernel_spmd

    x = np.asarray(x, dtype=np.float32)
    w1_bfly = np.asarray(w1_bfly, dtype=np.float32)
    w2_bfly = np.asarray(w2_bfly, dtype=np.float32)

    W1all, W2all = _build_host_weights(w1_bfly, w2_bfly)
    ident = np.eye(128, dtype=F16)
    nc = build_bass()
    in_maps = [
        {
            "x": np.ascontiguousarray(x[i * B_CORE:(i + 1) * B_CORE]),
            "w1": W1all,
            "w2": W2all,
            "ident": ident,
        }
        for i in range(NCORES)
    ]
    res = run_bass_kernel_spmd(
        nc, in_maps, core_ids=list(range(NCORES)), trace=_trace
    )
    outs = [_assemble_core(r["out"]) for r in res.results]
    full = np.concatenate(outs, axis=0)
    if _trace:
        return full, res
    return full


# revision 17
# speedup vs baseline: 322.0852x; 1.0274x over previous
"""Trainium2 Bass kernel for nn_ButterflyFFT (Monarch butterfly, N=4096, B=8192).

Math (per batch row b, viewing x[b] as a 64x64 matrix X with X[p,k]=x[b,p*64+k]):
  stage 1: for each column k: Y[:,k] = w1c[k] @ X[:,k]       (64x64 complex, X real)
  stage 2: for each row    l: Z[l,:] = w2c[l] @ Y[l,:]       (64x64 complex)
  output:  out[b, s*64+l] = Z[l,s]                            (complex64)

Device pipeline per core (B_core=1024, supertiles of BT=256):
  1. gather-DMA x -> T1[(h,p), (b0,k)] fp16 (cast in SWDGE DMA)
  2. stage 1, data-stationary fp16 matmuls: out (b, q2) -> G[b, ch, q*128+c*64+r]
  3. PE transpose per (l, ch): G-slice (b, (c r)) -> T2 (rc, b)
  4. stage 2, weights-stationary fp16 matmuls: O2 (c's*64+s, b)
  5. DMA out fp16 (l, cs, b); host reassembles complex64.
"""

import numpy as np

N = 4096
B = 8192
NCORES = 8
B_CORE = B // NCORES  # 1024
BT = 256              # supertile batch
NT = B_CORE // BT     # 4 supertiles
F16 = np.float16


def _build_host_weights(w1_bfly: np.ndarray, w2_bfly: np.ndarray):
    """W1all[64h+p, k*128 + c*64 + q] = w1_bfly[k,q,p,c]  (dup across h)
       W2all[c*64+r, l*128 + c'*64 + s] = stage-2 complex-matmul real form."""
    w1 = w1_bfly.astype(np.float32)              # (k, q, p, c)
    W1 = np.transpose(w1, (2, 0, 3, 1))          # (p, k, c, q)
    W1 = W1.reshape(64, 64 * 128).astype(F16)    # [p, k*128 + c*64 + q]
    W1all = np.concatenate([W1, W1], axis=0)     # dup rows for h=0/1

    w2r = w2_bfly[..., 0].astype(np.float32)     # (l, s, r)
    w2i = w2_bfly[..., 1].astype(np.float32)
    W2 = np.empty((2, 64, 64, 2, 64), dtype=np.float32)  # [c, r, l, c', s]
    W2[0, :, :, 0, :] = np.transpose(w2r, (2, 0, 1))     # rows r,    out re:  w2_re
    W2[1, :, :, 0, :] = -np.transpose(w2i, (2, 0, 1))    # rows 64+r, out re: -w2_im
    W2[0, :, :, 1, :] = np.transpose(w2i, (2, 0, 1))     # rows r,    out im:  w2_im
    W2[1, :, :, 1, :] = np.transpose(w2r, (2, 0, 1))     # rows 64+r, out im:  w2_re
    W2all = W2.reshape(128, 64 * 128).astype(F16)        # [c*64+r, l*128 + c'*64 + s]
    return np.ascontiguousarray(W1all), np.ascontiguousarray(W2all)


def build_bass(repeat=1):
    import concourse.bacc as bacc
    import concourse.mybir as mybir
    import concourse.tile as tile

    f16 = mybir.dt.float16
    f32 = mybir.dt.float32

    nc = bacc.Bacc("TRN2", target_bir_lowering=False)
    x = nc.dram_tensor("x", [B_CORE, N], f32, kind="ExternalInput")
    w1 = nc.dram_tensor("w1", [128, 64 * 128], f16, kind="ExternalInput")
    w2 = nc.dram_tensor("w2", [128, 64 * 128], f16, kind="ExternalInput")
    iddram = nc.dram_tensor("ident", [128, 128], f16, kind="ExternalInput")
    out = nc.dram_tensor("out", [64, 128, B_CORE], f16, kind="ExternalOutput")

    x_v = x[:, :].rearrange("(t h b0) (p k) -> t h p b0 k", h=2, b0=BT // 2, p=64)
    out_v = out[:, :, :].rearrange("L cs (t b) -> t cs L b", b=BT)

    with tile.TileContext(nc) as tc:
        with (
            tc.tile_pool(name="const", bufs=1) as constp,
            tc.tile_pool(name="t1", bufs=2) as t1p,
            tc.tile_pool(name="g", bufs=2) as gp,
            tc.tile_pool(name="t2s", bufs=4) as t2p,
            tc.tile_pool(name="outs", bufs=4) as outp,
            tc.tile_pool(name="po1", bufs=2, space="PSUM") as po1,
            tc.tile_pool(name="pt2", bufs=2, space="PSUM") as pt2,
            tc.tile_pool(name="po2", bufs=2, space="PSUM") as po2,
        ):
            # prefetch the first supertile's input before the weight tables
            T1_first = t1p.tile([128, (BT // 2) * 64], f16, tag="t1")
            T1f_4d = T1_first[:].rearrange("(h p) (b0 k) -> h p b0 k", h=2, k=64)
            for h in range(2):
                nc.gpsimd.dma_start(T1f_4d[h], x_v[0][h])
            W1t = constp.tile([128, 64 * 128], f16)
            nc.sync.dma_start(W1t[:], w1[:, :])
            ident = constp.tile([128, 128], f16)
            nc.sync.dma_start(ident[:], iddram[:, :])
            W2t = constp.tile([128, 64 * 128], f16)
            nc.sync.dma_start(W2t[:], w2[:, :])
            W1t_v = W1t[:].rearrange("(h p) f -> h p f", h=2)

            from contextlib import nullcontext
            rep_ctx = tc.For_i(0, repeat, 1) if repeat > 1 else nullcontext()
            with rep_ctx:
                for t in range(NT):
                    # ---- load T1[(h,p), (b0,k)] with fp32->fp16 cast (SWDGE) ----
                    if t == 0 and repeat == 1:
                        T1_4d = T1f_4d
                    else:
                        T1 = t1p.tile([128, (BT // 2) * 64], f16, tag="t1")
                        T1_4d = T1[:].rearrange("(h p) (b0 k) -> h p b0 k", h=2, k=64)
                        for h in range(2):
                            nc.gpsimd.dma_start(T1_4d[h], x_v[t][h])

                    # ---- stage 1 (data-stationary): G[b, ch, q*128+c*64+r] ----
                    G = gp.tile([128, 2, 64 * 128], f16)
                    G_5d = G[:].rearrange("B ch (q c r) -> B ch q c r", q=64, c=2)
                    for ch in range(2):
                        for kg2 in range(8):
                            O1 = po1.tile([128, 8, 128], f32)
                            for ksub in range(8):
                                k = kg2 * 8 + ksub
                                nc.tensor.matmul(
                                    O1[:, ksub, :],
                                    T1_4d[ch][:, :, k],                  # (64 p, 128 b0)
                                    W1t_v[ch][:, k * 128:(k + 1) * 128], # (64 p, 128 q2)
                                    start=True, stop=True,
                                )
                            # evac + cast f32->f16, (ksub,c,q)->(q,c,ksub)
                            src = O1[:].rearrange(
                                "B ksub (c q) -> B q c ksub", c=2)
                            dst = G_5d[:, ch, :, :, kg2 * 8:(kg2 + 1) * 8]
                            if kg2 % 4 == 3:
                                nc.vector.tensor_copy(dst, src)
                            else:
                                nc.scalar.copy(dst, src)

                    # ---- stage 2, l in quads: PE transposes -> T2s; pairs of mms ----
                    OUTS = None
                    T2s = None
                    for l0 in range(0, 64, 2):
                        grp = (t * 64 + l0) // 8
                        if l0 % 8 == 0:
                            OUTS = outp.tile([128, 8, BT], f16)
                        if l0 % 4 == 0:
                            Pt2 = pt2.tile([128, 8, 128], f16)
                            for lp in range(4):
                                l = l0 + lp
                                for ch in range(2):
                                    nc.tensor.transpose(
                                        Pt2[:, lp * 2 + ch, :],
                                        G[:, ch, l * 128:(l + 1) * 128], ident[:]
                                    )
                            T2s = t2p.tile([128, 4, 256], f16)
                            nc.vector.tensor_copy(T2s[:], Pt2[:])

                        O2 = po2.tile([128, 2, BT], f32)
                        for lp in range(2):
                            l = l0 + lp
                            nc.tensor.matmul(
                                O2[:, lp, :], W2t[:, l * 128:(l + 1) * 128],
                                T2s[:, l % 4, :],
                                start=True, stop=True,
                            )
                        if grp % 2 == 0:
                            nc.scalar.copy(OUTS[:, l0 % 8:l0 % 8 + 2, :], O2[:])
                        else:
                            nc.vector.tensor_copy(OUTS[:, l0 % 8:l0 % 8 + 2, :], O2[:])

                        if l0 % 8 == 6:
                            nc.sync.dma_start(out_v[t][:, l0 - 6:l0 + 2, :], OUTS[:])
    nc.compile()
    return nc


def _assemble_core(o: np.ndarray) -> np.ndarray:
    # o: (64 l, 128 cs, B_CORE) f16, cs = c*64+s  ->  (B_CORE, 4096) complex64
    a = o.reshape(64, 2, 64, B_CORE)                         # (l, c, s, b)
    a = np.ascontiguousarray(np.transpose(a, (3, 2, 0, 1)))  # (b, s, l, c)
    return a.astype(np.float32).view(np.complex64).reshape(B_CORE, N)


def kernel(x, w1_bfly, w2_bfly, perm, _trace=False):
    from concourse.bass_utils import run_bass_kernel_spmd

    x = np.asarray(x, dtype=np.float32)
    w1_bfly = np.asarray(w1_bfly, dtype=np.float32)
    w2_bfly = np.asarray(w2_bfly, dtype=np.float32)

    W1all, W2all = _build_host_weights(w1_bfly, w2_bfly)
    ident = np.eye(128, dtype=F16)
    nc = build_bass()
    in_maps = [
        {
            "x": np.ascontiguousarray(x[i * B_CORE:(i + 1) * B_CORE]),
            "w1": W1all,
            "w2": W2all,
            "ident": ident,
        }
        for i in range(NCORES)
    ]
    res = run_bass_kernel_spmd(
        nc, in_maps, core_ids=list(range(NCORES)), trace=_trace
    )
    outs = [_assemble_core(r["out"]) for r in res.results]
    full = np.concatenate(outs, axis=0)
    if _trace:
        return full, res
    return full
